# revision 1
# baseline (speedup 1.0000x reference)
"""JointEdgeSegLoss Trainium2 kernel.

Strategy (data-parallel over batch+rows, 8 cores):
  - core k handles image n=k//2, row-half h=k%2 (384 rows = 294912 pixels).
  - On-chip per core: log-softmax denominator via ACT exp + DVE strided
    reduce; per-(image,class) partial sums of lp = x_tgt - lse via fused
    scalar_tensor_tensor (is_equal * mult with free-dim accumulate);
    histogram counts via tensor_scalar accumulate; BCE partials via
    ACT (|x|, exp, ln1p, relu) + DVE dots.
  - Host combines tiny per-core partials in float64 (the "all-reduce").

Self-contained: hardcodes all shapes; only imports the runtime (concourse).
"""

import numpy as np

import concourse.bass as bass
import concourse.bacc as bacc
import concourse.mybir as mybir
import concourse.tile as tile
from concourse import bass_utils

F32 = mybir.dt.float32
I32 = mybir.dt.int32
BF16 = mybir.dt.bfloat16
ALU = mybir.AluOpType
ACTF = mybir.ActivationFunctionType

C = 19
N, H, W = 4, 768, 768
HW = H * W                      # pixels per image
NCORES = 8
M = N * HW // NCORES            # 294912 pixels per core (half an image)
P = 128
Q = M // P                      # 2304 free elements per partition
F = 384                         # pixels-per-partition per chunk
NCH = Q // F                    # 6 chunks
EDGE_THRESH = 0.8
IGNORE = 255.0

# accumulator slot layout (per chunk): 4 class-families of 19 + 3 bce slots
SL_S1 = 0          # sum (tgt==c) * lp            [19]
SL_S2 = SL_S1 + C  # sum (tgtv==c) * lp           [19]
SL_B1 = SL_S2 + C  # sum (tgt==c)                 [19]
SL_B2 = SL_B1 + C  # sum (tgtv==c)                [19]
SL_BCE = SL_B2 + C  # [sum t*bce, sum bce, sum t] [3]
SLOTS = SL_BCE + 3  # 79
NACC = NCH * SLOTS


def build_program():
    nc = bacc.Bacc("TRN2", target_bir_lowering=False, debug=False)

    xs = nc.dram_tensor("xs", [C, P, Q], F32, kind="ExternalInput")
    ts = nc.dram_tensor("ts", [P, Q], I32, kind="ExternalInput")
    es = nc.dram_tensor("es", [P, Q], F32, kind="ExternalInput")
    ms = nc.dram_tensor("ms", [P, Q], I32, kind="ExternalInput")
    acc_d = nc.dram_tensor("acc", [P, NACC], F32, kind="ExternalOutput")

    with tile.TileContext(nc) as tc:
        with (
            tc.tile_pool(name="xp", bufs=2) as xp,
            tc.tile_pool(name="ep", bufs=2) as ep,
            tc.tile_pool(name="lpp", bufs=2) as lpp,
            tc.tile_pool(name="mp", bufs=2) as mp,
            tc.tile_pool(name="sp", bufs=2) as sp,
            tc.tile_pool(name="cst", bufs=1) as cst,
        ):
            accT = cst.tile([P, NACC], F32, tag="acc")
            junk = cst.tile([P, F], F32, tag="junk")
            junk2 = cst.tile([P, F], F32, tag="junk2")

            for k in range(NCH):
                f0 = k * F

                X = xp.tile([P, C, F], F32, tag="X")
                nc.sync.dma_start(
                    X[:], xs.ap()[:, :, f0:f0 + F].transpose([1, 0, 2])
                )
                T = mp.tile([P, F], I32, tag="T")
                nc.sync.dma_start(T[:], ts.ap()[:, f0:f0 + F])
                E = mp.tile([P, F], F32, tag="E")
                nc.sync.dma_start(E[:], es.ap()[:, f0:f0 + F])
                Mm = mp.tile([P, F], I32, tag="Mm")
                nc.sync.dma_start(Mm[:], ms.ap()[:, f0:f0 + F])

                # ---- log-softmax denominator ----
                eb = ep.tile([P, C, F], BF16, tag="eb")
                nc.scalar.activation(eb[:], X[:], ACTF.Exp)
                s = sp.tile([P, F], F32, tag="s")
                nc.vector.tensor_reduce(
                    s[:], eb[:].transpose([0, 2, 1]), axis=mybir.AxisListType.X,
                    op=ALU.add,
                )
                lse = sp.tile([P, F], F32, tag="lse")
                nc.scalar.activation(lse[:], s[:], ACTF.Ln)

                # lp = x - lse (broadcast over channel)
                lp = lpp.tile([P, C, F], F32, tag="lp")
                nc.vector.scalar_tensor_tensor(
                    lp[:], X[:], 0.0,
                    lse[:].unsqueeze(1).broadcast_to([P, C, F]),
                    op0=ALU.add, op1=ALU.subtract,
                )

                # ---- masks ----
                Tf = sp.tile([P, F], F32, tag="Tf")
                nc.vector.tensor_copy(Tf[:], T[:])
                gt = sp.tile([P, F], F32, tag="gt")
                nc.vector.tensor_scalar(
                    gt[:], E[:], EDGE_THRESH, None, op0=ALU.is_gt
                )
                # Tv = gt ? Tf : 255  ==  (Tf - 255)*gt + 255
                Tvd = sp.tile([P, F], F32, tag="Tvd")
                nc.vector.scalar_tensor_tensor(
                    Tvd[:], Tf[:], -IGNORE, gt[:],
                    op0=ALU.add, op1=ALU.mult,
                )
                Tv = sp.tile([P, F], F32, tag="Tv")
                nc.vector.tensor_scalar(
                    Tv[:], Tvd[:], IGNORE, None, op0=ALU.add
                )

                base = k * SLOTS

                def slot(j):
                    return accT[:, base + j:base + j + 1]

                # ---- per-class families ----
                for c in range(C):
                    nc.vector.scalar_tensor_tensor(
                        junk[:], Tf[:], float(c), lp[:, c, :],
                        op0=ALU.is_equal, op1=ALU.mult,
                        accum_out=slot(SL_S1 + c),
                    )
                for c in range(C):
                    nc.vector.scalar_tensor_tensor(
                        junk[:], Tv[:], float(c), lp[:, c, :],
                        op0=ALU.is_equal, op1=ALU.mult,
                        accum_out=slot(SL_S2 + c),
                    )
                for c in range(C):
                    nc.vector.tensor_scalar(
                        junk2[:], Tf[:], float(c), None, op0=ALU.is_equal,
                        op1=ALU.add, accum_out=slot(SL_B1 + c),
                    )
                for c in range(C):
                    nc.vector.tensor_scalar(
                        junk2[:], Tv[:], float(c), None, op0=ALU.is_equal,
                        op1=ALU.add, accum_out=slot(SL_B2 + c),
                    )

                # ---- bce partials ----
                tm = sp.tile([P, F], F32, tag="tm")
                nc.vector.tensor_copy(tm[:], Mm[:])
                ab = sp.tile([P, F], F32, tag="ab")
                nc.scalar.activation(ab[:], E[:], ACTF.Abs)
                en = sp.tile([P, F], F32, tag="en")
                nc.scalar.activation(en[:], ab[:], ACTF.Exp, scale=-1.0)
                l1p = sp.tile([P, F], F32, tag="l1p")
                nc.scalar.activation(l1p[:], en[:], ACTF.Ln, bias=1.0)
                r = sp.tile([P, F], F32, tag="r")
                nc.scalar.activation(r[:], E[:], ACTF.Relu)
                # bce = r + l1p - E*t
                q = sp.tile([P, F], F32, tag="q")
                nc.vector.scalar_tensor_tensor(
                    q[:], E[:], 0.0, tm[:], op0=ALU.add, op1=ALU.mult
                )
                b1 = sp.tile([P, F], F32, tag="b1")
                nc.vector.scalar_tensor_tensor(
                    b1[:], r[:], 0.0, l1p[:], op0=ALU.add, op1=ALU.add
                )
                bce = sp.tile([P, F], F32, tag="bce")
                nc.vector.scalar_tensor_tensor(
                    bce[:], b1[:], 0.0, q[:], op0=ALU.add, op1=ALU.subtract,
                    accum_out=slot(SL_BCE + 1),
                )
                nc.vector.scalar_tensor_tensor(
                    junk[:], bce[:], 0.0, tm[:], op0=ALU.add, op1=ALU.mult,
                    accum_out=slot(SL_BCE + 0),
                )
                nc.vector.tensor_scalar(
                    junk2[:], tm[:], 0.0, None, op0=ALU.add,
                    op1=ALU.add, accum_out=slot(SL_BCE + 2),
                )

            nc.sync.dma_start(acc_d.ap()[:, :], accT[:])

    nc.finalize()
    return nc


_CACHE = {}


def _get_program():
    if "nc" not in _CACHE:
        _CACHE["nc"] = build_program()
    return _CACHE["nc"]


def make_in_maps(segin, edgein, segmask, edgemask):
    in_maps = []
    for k in range(NCORES):
        n, h = k // 2, k % 2
        rs = slice(h * (H // 2), (h + 1) * (H // 2))
        in_maps.append({
            "xs": np.ascontiguousarray(
                segin[n, :, rs, :].reshape(C, P, Q)),
            "ts": np.ascontiguousarray(
                segmask[n, rs, :].reshape(P, Q)),
            "es": np.ascontiguousarray(
                edgein[n, 0, rs, :].reshape(P, Q)),
            "ms": np.ascontiguousarray(
                edgemask[n, 0, rs, :].reshape(P, Q)),
        })
    return in_maps


def combine(acc_list):
    """acc_list: per-core [P, NACC] arrays -> final f32 scalar loss."""
    # per-core partial sums over partitions+chunks, in f64
    part = np.zeros((NCORES, SLOTS))
    for k in range(NCORES):
        a = acc_list[k].astype(np.float64).reshape(P, NCH, SLOTS)
        part[k] = a.sum(axis=(0, 1))

    seg_loss = 0.0
    att_loss = 0.0
    for n in range(N):
        p = part[2 * n] + part[2 * n + 1]
        S1 = p[SL_S1:SL_S1 + C]
        S2 = p[SL_S2:SL_S2 + C]
        bins = p[SL_B1:SL_B1 + C]
        bins2 = p[SL_B2:SL_B2 + C]

        w1 = (bins != 0) * (1.0 - bins / HW) + 1.0
        seg_loss += -(w1 * S1).sum() / (w1 * bins).sum()

        vsum = bins2.sum()
        w2 = (bins2 != 0) * (1.0 - bins2 / vsum) + 1.0
        att_loss += -(w2 * S2).sum() / (w2 * bins2).sum()

    tot = part.sum(axis=0)
    pos_bce, all_bce, pos_num = (
        tot[SL_BCE + 0], tot[SL_BCE + 1], tot[SL_BCE + 2])
    cnt = float(N * HW)
    neg_num = cnt - pos_num
    neg_bce = all_bce - pos_bce
    ssum = pos_num + neg_num
    edge_loss = (neg_num / ssum * pos_bce + pos_num / ssum * neg_bce) / cnt

    return np.float32(seg_loss + 0.3 * edge_loss + 0.1 * att_loss)


def run_cores(in_maps, trace=False, **kw):
    nc = _get_program()
    res = bass_utils.run_bass_kernel_spmd(
        nc, in_maps, core_ids=list(range(NCORES)), trace=trace, **kw
    )
    return res


def kernel(segin, edgein, segmask, edgemask):
    in_maps = make_in_maps(
        np.asarray(segin), np.asarray(edgein),
        np.asarray(segmask), np.asarray(edgemask))
    res = run_cores(in_maps)
    acc_list = [out["acc"] for out in res.results]
    return combine(acc_list)



# revision 2
# speedup vs baseline: 1.4332x; 1.4332x over previous
"""JointEdgeSegLoss Trainium2 kernel (v3: PE-matmul class sums, fp16).

Strategy (data-parallel over batch+rows, 8 cores):
  - core k handles image n=k//2, row-half h=k%2 (384 rows = 294912 pixels),
    laid out [P=128 partitions, Q=2304 free], 6 chunks of F=384.
  - fp16 on-chip pipeline: gpsimd cast-DMA loads x as fp16 (c-major Xh);
    ACT exp -> EB; DVE tree-add over classes -> S; ACT ln -> lse.
  - Per-(class,pixel) sums via the PE: per f-column the matmul
      stationary XT[:,f,:] = [x(19) | lse | 1]  (f-major, 6 f packed = 126)
      moving    OH[:,:,f]  = [onehot_t(19) | onehot_tv(19) | bce | tm | bce*tm]
    accumulates [126, 246] in PSUM over all 2304 columns. Host extracts
      T1[c]=sum (t==c) x[c],  L1[c]=sum (t==c) lse,  B1[c]=count(t==c)
    (and the tv family + bce sums) and combines S1 = T1 - L1 etc.
  - One-hots built on DVE at 2x: TT is_equal vs IOTA const, tv = t * gt.
  - Host combines tiny per-core partials in float64 (the "all-reduce").

Self-contained: hardcodes all shapes; only imports the runtime (concourse).
"""

import numpy as np

import concourse.bass as bass
import concourse.bacc as bacc
import concourse.mybir as mybir
import concourse.tile as tile
from concourse import bass_utils

F32 = mybir.dt.float32
I32 = mybir.dt.int32
FP16 = mybir.dt.float16
ALU = mybir.AluOpType
ACTF = mybir.ActivationFunctionType

C = 19
N, H, W = 4, 768, 768
HW = H * W
NCORES = 8
M = N * HW // NCORES            # 294912 pixels per core
P = 128
Q = M // P                      # 2304
F = 384                         # pixels-per-partition per chunk
NCH = Q // F                    # 6 chunks
PK = 6                          # f-columns packed per matmul
NST = C + 2                     # stationary slots: x[19] | lse | ones
NMV = 2 * C + 3                 # moving slots: oh_t | oh_tv | bce | tm | bce*tm
NRW = NST * PK                  # psum rows    126
NCL = NMV * PK                  # psum columns 246
EDGE_THRESH = 0.8


def build_program():
    nc = bacc.Bacc("TRN2", target_bir_lowering=False, debug=False)

    xs = nc.dram_tensor("xs", [C, P, Q], F32, kind="ExternalInput")
    ts = nc.dram_tensor("ts", [P, Q], I32, kind="ExternalInput")
    es = nc.dram_tensor("es", [P, Q], F32, kind="ExternalInput")
    ms = nc.dram_tensor("ms", [P, Q], I32, kind="ExternalInput")
    acc_d = nc.dram_tensor("acc", [NRW, NCL], F32, kind="ExternalOutput")

    with tile.TileContext(nc) as tc:
        with (
            tc.tile_pool(name="xp", bufs=2) as xp,
            tc.tile_pool(name="xtp", bufs=2) as xtp,
            tc.tile_pool(name="ebp", bufs=2) as ebp,
            tc.tile_pool(name="ohp", bufs=2) as ohp,
            tc.tile_pool(name="mp", bufs=2) as mp,
            tc.tile_pool(name="sp", bufs=2) as sp,
            tc.tile_pool(name="cst", bufs=1) as cst,
            tc.tile_pool(name="ps", bufs=1, space=bass.MemorySpace.PSUM) as psp,
        ):
            IOTA = cst.tile([P, C, F], FP16, tag="iota")
            for c in range(C):
                nc.gpsimd.memset(IOTA[:, c, :], float(c))

            acc = psp.tile([NRW, NCL], F32, tag="acc")

            for k in range(NCH):
                f0 = k * F

                Xh = xp.tile([P, C, F], FP16, tag="Xh")
                nc.gpsimd.dma_start(
                    Xh[:], xs.ap()[:, :, f0:f0 + F].transpose([1, 0, 2])
                )
                T = mp.tile([P, F], I32, tag="T")
                nc.sync.dma_start(T[:], ts.ap()[:, f0:f0 + F])
                E = mp.tile([P, F], FP16, tag="E")
                nc.gpsimd.dma_start(E[:], es.ap()[:, f0:f0 + F])
                Mm = mp.tile([P, F], I32, tag="Mm")
                nc.sync.dma_start(Mm[:], ms.ap()[:, f0:f0 + F])

                # ---- stationary tile: x (f-major) | lse | ones ----
                XT = xtp.tile([P, F, NST], FP16, tag="XT")
                nc.gpsimd.tensor_copy(
                    XT[:, :, 0:C], Xh[:].transpose([0, 2, 1]))
                nc.gpsimd.memset(XT[:, :, C + 1:NST], 1.0)

                # ---- log-softmax denominator ----
                EB = ebp.tile([P, C, F], FP16, tag="EB")
                nc.scalar.activation(EB[:], Xh[:], ACTF.Exp)
                # tree-sum over classes into EB[:, 0, :]
                nc.vector.tensor_tensor(
                    out=EB[:, 0:9, :], in0=EB[:, 0:9, :], in1=EB[:, 9:18, :],
                    op=ALU.add)
                nc.vector.tensor_tensor(
                    out=EB[:, 0:4, :], in0=EB[:, 0:4, :], in1=EB[:, 4:8, :],
                    op=ALU.add)
                nc.vector.tensor_tensor(
                    out=EB[:, 0:2, :], in0=EB[:, 0:2, :], in1=EB[:, 2:4, :],
                    op=ALU.add)
                nc.vector.tensor_tensor(
                    out=EB[:, 0:1, :], in0=EB[:, 0:1, :], in1=EB[:, 1:2, :],
                    op=ALU.add)
                nc.vector.tensor_tensor(
                    out=EB[:, 0:1, :], in0=EB[:, 0:1, :], in1=EB[:, 8:9, :],
                    op=ALU.add)
                nc.vector.tensor_tensor(
                    out=EB[:, 0:1, :], in0=EB[:, 0:1, :], in1=EB[:, 18:19, :],
                    op=ALU.add)
                # lse -> XT slot 19 (strided column write on ACT)
                nc.scalar.activation(
                    XT[:, :, C:C + 1], EB[:, 0:1, :].transpose([0, 2, 1]),
                    ACTF.Ln)

                # ---- one-hots (c-major) ----
                Tf = sp.tile([P, F], FP16, tag="Tf")
                nc.vector.tensor_copy(Tf[:], T[:])
                gt = sp.tile([P, F], FP16, tag="gt")
                nc.vector.tensor_scalar(
                    gt[:], E[:], EDGE_THRESH, None, op0=ALU.is_gt)

                OH = ohp.tile([P, NMV, F], FP16, tag="OH")
                nc.vector.tensor_tensor(
                    out=OH[:, 0:C, :],
                    in0=Tf[:].unsqueeze(1).broadcast_to([P, C, F]),
                    in1=IOTA[:], op=ALU.is_equal)
                nc.vector.tensor_tensor(
                    out=OH[:, C:2 * C, :], in0=OH[:, 0:C, :],
                    in1=gt[:].unsqueeze(1).broadcast_to([P, C, F]),
                    op=ALU.mult)

                # ---- bce terms into OH slots 38..40 ----
                tm = sp.tile([P, F], FP16, tag="tm")
                nc.vector.tensor_copy(tm[:], Mm[:])
                ab = sp.tile([P, F], FP16, tag="ab")
                nc.scalar.activation(ab[:], E[:], ACTF.Abs)
                en = sp.tile([P, F], FP16, tag="en")
                nc.scalar.activation(en[:], ab[:], ACTF.Exp, scale=-1.0)
                l1p = sp.tile([P, F], FP16, tag="l1p")
                nc.scalar.activation(l1p[:], en[:], ACTF.Ln, bias=1.0)
                r = sp.tile([P, F], FP16, tag="r")
                nc.scalar.activation(r[:], E[:], ACTF.Relu)
                q = sp.tile([P, F], FP16, tag="q")
                nc.vector.tensor_tensor(out=q[:], in0=E[:], in1=tm[:],
                                        op=ALU.mult)
                b1 = sp.tile([P, F], FP16, tag="b1")
                nc.vector.tensor_tensor(out=b1[:], in0=r[:], in1=l1p[:],
                                        op=ALU.add)
                nc.vector.tensor_tensor(out=OH[:, 2 * C, :], in0=b1[:],
                                        in1=q[:], op=ALU.subtract)
                nc.vector.tensor_copy(OH[:, 2 * C + 1, :], tm[:])
                nc.vector.tensor_tensor(out=OH[:, 2 * C + 2, :],
                                        in0=OH[:, 2 * C, :], in1=tm[:],
                                        op=ALU.mult)

                # ---- PE: packed matmuls accumulate [NRW, NCL] ----
                for i in range(F // PK):
                    fa = i * PK
                    nc.tensor.matmul(
                        acc[:, :],
                        XT[:, fa:fa + PK, :],
                        OH[:, :, fa:fa + PK],
                        start=(k == 0 and i == 0),
                        stop=(k == NCH - 1 and i == F // PK - 1),
                    )

            res = cst.tile([NRW, NCL], F32, tag="res")
            nc.vector.tensor_copy(res[:], acc[:])
            nc.sync.dma_start(acc_d.ap()[:, :], res[:])

    nc.finalize()
    return nc


_CACHE = {}


def _get_program():
    if "nc" not in _CACHE:
        _CACHE["nc"] = build_program()
    return _CACHE["nc"]


def make_in_maps(segin, edgein, segmask, edgemask):
    in_maps = []
    for k in range(NCORES):
        n, h = k // 2, k % 2
        rs = slice(h * (H // 2), (h + 1) * (H // 2))
        in_maps.append({
            "xs": np.ascontiguousarray(
                segin[n, :, rs, :].reshape(C, P, Q)),
            "ts": np.ascontiguousarray(
                segmask[n, rs, :].reshape(P, Q)),
            "es": np.ascontiguousarray(
                edgein[n, 0, rs, :].reshape(P, Q)),
            "ms": np.ascontiguousarray(
                edgemask[n, 0, rs, :].reshape(P, Q)),
        })
    return in_maps


def extract_core(acc):
    """acc: [NRW, NCL] f32 psum dump -> dict of per-core partial vectors."""
    a = acc.astype(np.float64).reshape(PK, NST, NMV, PK)
    # valid entries: stationary f == moving f
    v = np.einsum("fsmf->sm", a)          # [NST, NMV]
    T1 = np.array([v[c, c] for c in range(C)])
    T2 = np.array([v[c, C + c] for c in range(C)])
    L1 = v[C, 0:C]
    L2 = v[C, C:2 * C]
    B1 = v[C + 1, 0:C]
    B2 = v[C + 1, C:2 * C]
    bce_sum = v[C + 1, 2 * C]
    t_sum = v[C + 1, 2 * C + 1]
    bce_t_sum = v[C + 1, 2 * C + 2]
    return {
        "S1": T1 - L1, "S2": T2 - L2, "B1": B1, "B2": B2,
        "bce": bce_sum, "t": t_sum, "bce_t": bce_t_sum,
    }


def combine(acc_list):
    """acc_list: per-core [NRW, NCL] arrays -> final f32 scalar loss."""
    parts = [extract_core(a) for a in acc_list]

    seg_loss = 0.0
    att_loss = 0.0
    for n in range(N):
        pa, pb = parts[2 * n], parts[2 * n + 1]
        S1 = pa["S1"] + pb["S1"]
        S2 = pa["S2"] + pb["S2"]
        bins = pa["B1"] + pb["B1"]
        bins2 = pa["B2"] + pb["B2"]

        w1 = (bins != 0) * (1.0 - bins / HW) + 1.0
        seg_loss += -(w1 * S1).sum() / (w1 * bins).sum()

        vsum = bins2.sum()
        w2 = (bins2 != 0) * (1.0 - bins2 / vsum) + 1.0
        att_loss += -(w2 * S2).sum() / (w2 * bins2).sum()

    pos_bce = sum(p["bce_t"] for p in parts)
    all_bce = sum(p["bce"] for p in parts)
    pos_num = sum(p["t"] for p in parts)
    cnt = float(N * HW)
    neg_num = cnt - pos_num
    neg_bce = all_bce - pos_bce
    ssum = pos_num + neg_num
    edge_loss = (neg_num / ssum * pos_bce + pos_num / ssum * neg_bce) / cnt

    return np.float32(seg_loss + 0.3 * edge_loss + 0.1 * att_loss)


def run_cores(in_maps, trace=False, **kw):
    nc = _get_program()
    res = bass_utils.run_bass_kernel_spmd(
        nc, in_maps, core_ids=list(range(NCORES)), trace=trace, **kw
    )
    return res


def kernel(segin, edgein, segmask, edgemask):
    in_maps = make_in_maps(
        np.asarray(segin), np.asarray(edgein),
        np.asarray(segmask), np.asarray(edgemask))
    res = run_cores(in_maps)
    acc_list = [out["acc"] for out in res.results]
    return combine(acc_list)


# revision 3
# speedup vs baseline: 2.3812x; 1.6615x over previous
"""JointEdgeSegLoss Trainium2 kernel (v4: PE-matmul class sums, fp16,
host-side f-major layout).

Strategy (data-parallel over batch+rows, 8 cores):
  - core k handles image n=k//2, row-half h=k%2 (294912 pixels), laid out
    [P=128 partitions, Q=2304 free], 6 chunks of F=384.
  - Host pre-packs x per core as fp16 [P, Q, 21] (f-major): slots 0..18 =
    the 19 class logits, slot 19 = 0 (device writes lse there), slot 20 = 1.
  - Device: ACT exp -> EB (f-major); DVE tree-add over classes -> S;
    ACT ln -> lse into slot 19.
  - All per-(class,pixel) sums via the PE: per f-column
      stationary XF[:,f,:] = [x(19) | lse | 1]   (6 f packed = 126 cols)
      moving    OH[:,:,f]  = [onehot_t | onehot_tv | bce | tm | bce*tm]
    accumulate [126, 246] in PSUM over all 2304 columns. Host extracts
      T1[c]=sum (t==c) x[c], L1[c]=sum (t==c) lse, B1[c]=count(t==c)
    (plus tv family and bce sums), then S1 = T1 - L1 etc.
  - One-hots on DVE at 2x fp16: TT is_equal vs IOTA const; oh_tv = oh_t*gt.
  - Host combines tiny per-core partials in float64 (the "all-reduce").

Self-contained: hardcodes all shapes; only imports the runtime (concourse).
"""

import numpy as np

import concourse.bass as bass
import concourse.bacc as bacc
import concourse.mybir as mybir
import concourse.tile as tile
from concourse import bass_utils

F32 = mybir.dt.float32
I32 = mybir.dt.int32
FP16 = mybir.dt.float16
ALU = mybir.AluOpType
ACTF = mybir.ActivationFunctionType

C = 19
N, H, W = 4, 768, 768
HW = H * W
NCORES = 8
M = N * HW // NCORES            # 294912 pixels per core
P = 128
Q = M // P                      # 2304
F = 384                         # pixels-per-partition per chunk
NCH = Q // F                    # 6 chunks
PK = 6                          # f-columns packed per matmul
NST = C + 2                     # stationary slots: x[19] | lse | ones
NMV = 2 * C + 3                 # moving slots: oh_t | oh_tv | bce | tm | bce*tm
NRW = NST * PK                  # psum rows    126
NCL = NMV * PK                  # psum columns 246
EDGE_THRESH = 0.8


def build_program():
    nc = bacc.Bacc("TRN2", target_bir_lowering=False, debug=False)

    xs = nc.dram_tensor("xs", [P, Q, NST], FP16, kind="ExternalInput")
    ts = nc.dram_tensor("ts", [P, Q], I32, kind="ExternalInput")
    es = nc.dram_tensor("es", [P, Q], F32, kind="ExternalInput")
    ms = nc.dram_tensor("ms", [P, Q], I32, kind="ExternalInput")
    acc_d = nc.dram_tensor("acc", [NRW, NCL], F32, kind="ExternalOutput")

    with tile.TileContext(nc) as tc:
        with (
            tc.tile_pool(name="xp", bufs=2) as xp,
            tc.tile_pool(name="ebp", bufs=2) as ebp,
            tc.tile_pool(name="ohp", bufs=2) as ohp,
            tc.tile_pool(name="mp", bufs=2) as mp,
            tc.tile_pool(name="sp", bufs=2) as sp,
            tc.tile_pool(name="cst", bufs=1) as cst,
            tc.tile_pool(name="ps", bufs=1, space=bass.MemorySpace.PSUM) as psp,
        ):
            IOTA = cst.tile([P, C, F], FP16, tag="iota")
            for c in range(C):
                nc.gpsimd.memset(IOTA[:, c, :], float(c))

            acc = psp.tile([NRW, NCL], F32, tag="acc")

            for k in range(NCH):
                f0 = k * F

                XF = xp.tile([P, F, NST], FP16, tag="XF")
                nc.sync.dma_start(XF[:], xs.ap()[:, f0:f0 + F, :])
                T = mp.tile([P, F], I32, tag="T")
                nc.sync.dma_start(T[:], ts.ap()[:, f0:f0 + F])
                E = mp.tile([P, F], F32, tag="E")
                nc.sync.dma_start(E[:], es.ap()[:, f0:f0 + F])
                Mm = mp.tile([P, F], I32, tag="Mm")
                nc.sync.dma_start(Mm[:], ms.ap()[:, f0:f0 + F])

                # ---- log-softmax denominator (f-major) ----
                EB = ebp.tile([P, F, C], FP16, tag="EB")
                nc.scalar.activation(EB[:], XF[:, :, 0:C], ACTF.Exp)
                # tree-sum over classes into EB[:, :, 0]
                nc.vector.tensor_tensor(
                    out=EB[:, :, 0:9], in0=EB[:, :, 0:9], in1=EB[:, :, 9:18],
                    op=ALU.add)
                nc.vector.tensor_tensor(
                    out=EB[:, :, 0:4], in0=EB[:, :, 0:4], in1=EB[:, :, 4:8],
                    op=ALU.add)
                nc.vector.tensor_tensor(
                    out=EB[:, :, 0:2], in0=EB[:, :, 0:2], in1=EB[:, :, 2:4],
                    op=ALU.add)
                nc.vector.tensor_tensor(
                    out=EB[:, :, 0:1], in0=EB[:, :, 0:1], in1=EB[:, :, 1:2],
                    op=ALU.add)
                nc.vector.tensor_tensor(
                    out=EB[:, :, 0:1], in0=EB[:, :, 0:1], in1=EB[:, :, 8:9],
                    op=ALU.add)
                nc.vector.tensor_tensor(
                    out=EB[:, :, 0:1], in0=EB[:, :, 0:1], in1=EB[:, :, 18:19],
                    op=ALU.add)
                # lse -> XF slot 19 (strided column on ACT)
                nc.scalar.activation(
                    XF[:, :, C:C + 1], EB[:, :, 0:1], ACTF.Ln)

                # ---- one-hots (c-major) ----
                Tf = sp.tile([P, F], FP16, tag="Tf")
                nc.vector.tensor_copy(Tf[:], T[:])
                gt = sp.tile([P, F], FP16, tag="gt")
                nc.vector.tensor_scalar(
                    gt[:], E[:], EDGE_THRESH, None, op0=ALU.is_gt)

                OH = ohp.tile([P, NMV, F], FP16, tag="OH")
                nc.vector.tensor_tensor(
                    out=OH[:, 0:C, :],
                    in0=Tf[:].unsqueeze(1).broadcast_to([P, C, F]),
                    in1=IOTA[:], op=ALU.is_equal)
                nc.vector.tensor_tensor(
                    out=OH[:, C:2 * C, :], in0=OH[:, 0:C, :],
                    in1=gt[:].unsqueeze(1).broadcast_to([P, C, F]),
                    op=ALU.mult)

                # ---- bce terms into OH slots 38..40 ----
                tm = sp.tile([P, F], FP16, tag="tm")
                nc.vector.tensor_copy(tm[:], Mm[:])
                ab = sp.tile([P, F], FP16, tag="ab")
                nc.scalar.activation(ab[:], E[:], ACTF.Abs)
                en = sp.tile([P, F], FP16, tag="en")
                nc.scalar.activation(en[:], ab[:], ACTF.Exp, scale=-1.0)
                l1p = sp.tile([P, F], FP16, tag="l1p")
                nc.scalar.activation(l1p[:], en[:], ACTF.Ln, bias=1.0)
                r = sp.tile([P, F], FP16, tag="r")
                nc.scalar.activation(r[:], E[:], ACTF.Relu)
                q = sp.tile([P, F], FP16, tag="q")
                nc.vector.tensor_tensor(out=q[:], in0=E[:], in1=tm[:],
                                        op=ALU.mult)
                b1 = sp.tile([P, F], FP16, tag="b1")
                nc.vector.tensor_tensor(out=b1[:], in0=r[:], in1=l1p[:],
                                        op=ALU.add)
                nc.vector.tensor_tensor(out=OH[:, 2 * C, :], in0=b1[:],
                                        in1=q[:], op=ALU.subtract)
                nc.vector.tensor_copy(OH[:, 2 * C + 1, :], tm[:])
                nc.vector.tensor_tensor(out=OH[:, 2 * C + 2, :],
                                        in0=OH[:, 2 * C, :], in1=tm[:],
                                        op=ALU.mult)

                # ---- PE: packed matmuls accumulate [NRW, NCL] ----
                for i in range(F // PK):
                    fa = i * PK
                    nc.tensor.matmul(
                        acc[:, :],
                        XF[:, fa:fa + PK, :],
                        OH[:, :, fa:fa + PK],
                        start=(k == 0 and i == 0),
                        stop=(k == NCH - 1 and i == F // PK - 1),
                    )

            res = cst.tile([NRW, NCL], F32, tag="res")
            nc.vector.tensor_copy(res[:], acc[:])
            nc.sync.dma_start(acc_d.ap()[:, :], res[:])

    nc.finalize()
    return nc


_CACHE = {}


def _get_program():
    if "nc" not in _CACHE:
        _CACHE["nc"] = build_program()
    return _CACHE["nc"]


def make_in_maps(segin, edgein, segmask, edgemask):
    segin = np.asarray(segin)
    in_maps = []
    for k in range(NCORES):
        n, h = k // 2, k % 2
        rs = slice(h * (H // 2), (h + 1) * (H // 2))
        # [C, P, Q] -> f-major [P, Q, C], pad to NST slots (lse=0, ones=1)
        xc = segin[n, :, rs, :].reshape(C, P, Q)
        xf = np.zeros((P, Q, NST), dtype=np.float16)
        xf[:, :, 0:C] = xc.transpose(1, 2, 0)
        xf[:, :, C + 1] = 1.0
        in_maps.append({
            "xs": xf,
            "ts": np.ascontiguousarray(
                segmask[n, rs, :].reshape(P, Q)),
            "es": np.ascontiguousarray(
                edgein[n, 0, rs, :].reshape(P, Q)),
            "ms": np.ascontiguousarray(
                edgemask[n, 0, rs, :].reshape(P, Q)),
        })
    return in_maps


def extract_core(acc):
    """acc: [NRW, NCL] f32 psum dump -> dict of per-core partial sums."""
    a = acc.astype(np.float64).reshape(PK, NST, NMV, PK)
    v = np.einsum("fsmf->sm", a)          # [NST, NMV], diag over packed f
    T1 = np.array([v[c, c] for c in range(C)])
    T2 = np.array([v[c, C + c] for c in range(C)])
    L1 = v[C, 0:C]
    L2 = v[C, C:2 * C]
    B1 = v[C + 1, 0:C]
    B2 = v[C + 1, C:2 * C]
    bce_sum = v[C + 1, 2 * C]
    t_sum = v[C + 1, 2 * C + 1]
    bce_t_sum = v[C + 1, 2 * C + 2]
    return {
        "S1": T1 - L1, "S2": T2 - L2, "B1": B1, "B2": B2,
        "bce": bce_sum, "t": t_sum, "bce_t": bce_t_sum,
    }


def combine(acc_list):
    """acc_list: per-core [NRW, NCL] arrays -> final f32 scalar loss."""
    parts = [extract_core(a) for a in acc_list]

    seg_loss = 0.0
    att_loss = 0.0
    for n in range(N):
        pa, pb = parts[2 * n], parts[2 * n + 1]
        S1 = pa["S1"] + pb["S1"]
        S2 = pa["S2"] + pb["S2"]
        bins = pa["B1"] + pb["B1"]
        bins2 = pa["B2"] + pb["B2"]

        w1 = (bins != 0) * (1.0 - bins / HW) + 1.0
        seg_loss += -(w1 * S1).sum() / (w1 * bins).sum()

        vsum = bins2.sum()
        w2 = (bins2 != 0) * (1.0 - bins2 / vsum) + 1.0
        att_loss += -(w2 * S2).sum() / (w2 * bins2).sum()

    pos_bce = sum(p["bce_t"] for p in parts)
    all_bce = sum(p["bce"] for p in parts)
    pos_num = sum(p["t"] for p in parts)
    cnt = float(N * HW)
    neg_num = cnt - pos_num
    neg_bce = all_bce - pos_bce
    ssum = pos_num + neg_num
    edge_loss = (neg_num / ssum * pos_bce + pos_num / ssum * neg_bce) / cnt

    return np.float32(seg_loss + 0.3 * edge_loss + 0.1 * att_loss)


def run_cores(in_maps, trace=False, **kw):
    nc = _get_program()
    res = bass_utils.run_bass_kernel_spmd(
        nc, in_maps, core_ids=list(range(NCORES)), trace=trace, **kw
    )
    return res


def kernel(segin, edgein, segmask, edgemask):
    in_maps = make_in_maps(
        np.asarray(segin), np.asarray(edgein),
        np.asarray(segmask), np.asarray(edgemask))
    res = run_cores(in_maps)
    acc_list = [out["acc"] for out in res.results]
    return combine(acc_list)


# revision 7
# speedup vs baseline: 3.1635x; 1.3285x over previous
"""JointEdgeSegLoss Trainium2 kernel (v4: PE-matmul class sums, fp16,
host-side f-major layout).

Strategy (data-parallel over batch+rows, 8 cores):
  - core k handles image n=k//2, row-half h=k%2 (294912 pixels), laid out
    [P=128 partitions, Q=2304 free], 6 chunks of F=384.
  - Host pre-packs x per core as fp16 [P, Q, 21] (f-major): slots 0..18 =
    the 19 class logits, slot 19 = 0 (device writes lse there), slot 20 = 1.
  - Device: ACT exp -> EB (f-major); DVE tree-add over classes -> S;
    ACT ln -> lse into slot 19.
  - All per-(class,pixel) sums via the PE: per f-column
      stationary XF[:,f,:] = [x(19) | lse | 1]   (6 f packed = 126 cols)
      moving    OH[:,:,f]  = [onehot_t | onehot_tv | bce | tm | bce*tm]
    accumulate [126, 246] in PSUM over all 2304 columns. Host extracts
      T1[c]=sum (t==c) x[c], L1[c]=sum (t==c) lse, B1[c]=count(t==c)
    (plus tv family and bce sums), then S1 = T1 - L1 etc.
  - One-hots on DVE at 2x fp16: TT is_equal vs IOTA const; oh_tv = oh_t*gt.
  - Host combines tiny per-core partials in float64 (the "all-reduce").

Self-contained: hardcodes all shapes; only imports the runtime (concourse).
"""

import numpy as np

import concourse.bass as bass
import concourse.bacc as bacc
import concourse.mybir as mybir
import concourse.tile as tile
from concourse import bass_utils

F32 = mybir.dt.float32
I32 = mybir.dt.int32
FP16 = mybir.dt.float16
ALU = mybir.AluOpType
ACTF = mybir.ActivationFunctionType

C = 19
N, H, W = 4, 768, 768
HW = H * W
NCORES = 8
M = N * HW // NCORES            # 294912 pixels per core
P = 128
Q = M // P                      # 2304
F = 384                         # pixels-per-partition per chunk
NCH = Q // F                    # 6 chunks
PK = 6                          # f-columns packed per matmul
NST = C + 2                     # stationary slots: x[19] | lse | ones
NMV = 2 * C + 3                 # moving slots: oh_t | oh_tv | bce | tm | bce*tm
NRW = NST * PK                  # psum rows    126
NCL = NMV * PK                  # psum columns 246
EDGE_THRESH = 0.8


def build_program():
    nc = bacc.Bacc("TRN2", target_bir_lowering=False, debug=False)

    xs = nc.dram_tensor("xs", [P, Q, NST], FP16, kind="ExternalInput")
    xc = nc.dram_tensor("xc", [P, C, Q], FP16, kind="ExternalInput")
    ts = nc.dram_tensor("ts", [P, Q], I32, kind="ExternalInput")
    es = nc.dram_tensor("es", [P, Q], F32, kind="ExternalInput")
    ms = nc.dram_tensor("ms", [P, Q], I32, kind="ExternalInput")
    acc_d = nc.dram_tensor("acc", [NRW, NCL], F32, kind="ExternalOutput")

    with tile.TileContext(nc) as tc:
        with (
            tc.tile_pool(name="xp", bufs=2) as xp,
            tc.tile_pool(name="ebp", bufs=2) as ebp,
            tc.tile_pool(name="ohp", bufs=2) as ohp,
            tc.tile_pool(name="mp", bufs=2) as mp,
            tc.tile_pool(name="sp", bufs=2) as sp,
            tc.tile_pool(name="cst", bufs=1) as cst,
            tc.tile_pool(name="ps", bufs=1, space=bass.MemorySpace.PSUM) as psp,
        ):
            IOTA = cst.tile([P, C, F], FP16, tag="iota")
            for c in range(C):
                nc.gpsimd.memset(IOTA[:, c, :], float(c))

            acc = psp.tile([NRW, NCL], F32, tag="acc")

            for k in range(NCH):
                f0 = k * F

                XF = xp.tile([P, F, NST], FP16, tag="XF")
                nc.sync.dma_start(XF[:], xs.ap()[:, f0:f0 + F, :])
                XC = xp.tile([P, C, F], FP16, tag="XC")
                nc.sync.dma_start(XC[:], xc.ap()[:, :, f0:f0 + F])
                T = mp.tile([P, F], I32, tag="T")
                nc.sync.dma_start(T[:], ts.ap()[:, f0:f0 + F])
                E = mp.tile([P, F], F32, tag="E")
                nc.sync.dma_start(E[:], es.ap()[:, f0:f0 + F])
                Mm = mp.tile([P, F], I32, tag="Mm")
                nc.sync.dma_start(Mm[:], ms.ap()[:, f0:f0 + F])

                # ---- log-softmax denominator (c-major, contiguous tree) ----
                EB = ebp.tile([P, C, F], FP16, tag="EB")
                nc.scalar.activation(EB[:], XC[:], ACTF.Exp)
                nc.vector.tensor_tensor(
                    out=EB[:, 0:9, :], in0=EB[:, 0:9, :], in1=EB[:, 9:18, :],
                    op=ALU.add)
                nc.vector.tensor_tensor(
                    out=EB[:, 0:4, :], in0=EB[:, 0:4, :], in1=EB[:, 4:8, :],
                    op=ALU.add)
                nc.vector.tensor_tensor(
                    out=EB[:, 0:2, :], in0=EB[:, 0:2, :], in1=EB[:, 2:4, :],
                    op=ALU.add)
                nc.vector.tensor_tensor(
                    out=EB[:, 0:1, :], in0=EB[:, 0:1, :], in1=EB[:, 1:2, :],
                    op=ALU.add)
                nc.vector.tensor_tensor(
                    out=EB[:, 0:1, :], in0=EB[:, 0:1, :], in1=EB[:, 8:9, :],
                    op=ALU.add)
                nc.vector.tensor_tensor(
                    out=EB[:, 0:1, :], in0=EB[:, 0:1, :], in1=EB[:, 18:19, :],
                    op=ALU.add)
                # lse -> XF slot 19 (strided column on ACT)
                nc.scalar.activation(
                    XF[:, :, C:C + 1], EB[:, 0:1, :].transpose([0, 2, 1]),
                    ACTF.Ln)

                # ---- one-hots (c-major) ----
                Tf = sp.tile([P, F], FP16, tag="Tf")
                nc.vector.tensor_copy(Tf[:], T[:])
                gt = sp.tile([P, F], FP16, tag="gt")
                nc.vector.tensor_scalar(
                    gt[:], E[:], EDGE_THRESH, None, op0=ALU.is_gt)

                OH = ohp.tile([P, NMV, F], FP16, tag="OH")
                nc.vector.tensor_tensor(
                    out=OH[:, 0:C, :],
                    in0=Tf[:].unsqueeze(1).broadcast_to([P, C, F]),
                    in1=IOTA[:], op=ALU.is_equal)
                nc.vector.tensor_tensor(
                    out=OH[:, C:2 * C, :], in0=OH[:, 0:C, :],
                    in1=gt[:].unsqueeze(1).broadcast_to([P, C, F]),
                    op=ALU.mult)

                # ---- bce terms into OH slots 38..40 ----
                tm = sp.tile([P, F], FP16, tag="tm")
                nc.vector.tensor_copy(tm[:], Mm[:])
                ab = sp.tile([P, F], FP16, tag="ab")
                nc.scalar.activation(ab[:], E[:], ACTF.Abs)
                en = sp.tile([P, F], FP16, tag="en")
                nc.scalar.activation(en[:], ab[:], ACTF.Exp, scale=-1.0)
                l1p = sp.tile([P, F], FP16, tag="l1p")
                nc.scalar.activation(l1p[:], en[:], ACTF.Ln, bias=1.0)
                r = sp.tile([P, F], FP16, tag="r")
                nc.scalar.activation(r[:], E[:], ACTF.Relu)
                q = sp.tile([P, F], FP16, tag="q")
                nc.vector.tensor_tensor(out=q[:], in0=E[:], in1=tm[:],
                                        op=ALU.mult)
                b1 = sp.tile([P, F], FP16, tag="b1")
                nc.vector.tensor_tensor(out=b1[:], in0=r[:], in1=l1p[:],
                                        op=ALU.add)
                nc.vector.tensor_tensor(out=OH[:, 2 * C, :], in0=b1[:],
                                        in1=q[:], op=ALU.subtract)
                nc.vector.tensor_copy(OH[:, 2 * C + 1, :], tm[:])
                nc.vector.tensor_tensor(out=OH[:, 2 * C + 2, :],
                                        in0=OH[:, 2 * C, :], in1=tm[:],
                                        op=ALU.mult)

                # ---- PE: packed matmuls accumulate [NRW, NCL] ----
                for i in range(F // PK):
                    fa = i * PK
                    nc.tensor.matmul(
                        acc[:, :],
                        XF[:, fa:fa + PK, :],
                        OH[:, :, fa:fa + PK],
                        start=(k == 0 and i == 0),
                        stop=(k == NCH - 1 and i == F // PK - 1),
                    )

            res = cst.tile([NRW, NCL], F32, tag="res")
            nc.vector.tensor_copy(res[:], acc[:])
            nc.sync.dma_start(acc_d.ap()[:, :], res[:])

    nc.finalize()
    return nc


_CACHE = {}


def _get_program():
    if "nc" not in _CACHE:
        _CACHE["nc"] = build_program()
    return _CACHE["nc"]


def make_in_maps(segin, edgein, segmask, edgemask):
    segin = np.asarray(segin)
    in_maps = []
    for k in range(NCORES):
        n, h = k // 2, k % 2
        rs = slice(h * (H // 2), (h + 1) * (H // 2))
        # [C, P, Q] -> f-major [P, Q, C], pad to NST slots (lse=0, ones=1)
        xcm = segin[n, :, rs, :].reshape(C, P, Q)
        xf = np.zeros((P, Q, NST), dtype=np.float16)
        xf[:, :, 0:C] = xcm.transpose(1, 2, 0)
        xf[:, :, C + 1] = 1.0
        in_maps.append({
            "xs": xf,
            "xc": np.ascontiguousarray(
                xcm.transpose(1, 0, 2).astype(np.float16)),
            "ts": np.ascontiguousarray(
                segmask[n, rs, :].reshape(P, Q)),
            "es": np.ascontiguousarray(
                edgein[n, 0, rs, :].reshape(P, Q)),
            "ms": np.ascontiguousarray(
                edgemask[n, 0, rs, :].reshape(P, Q)),
        })
    return in_maps


def extract_core(acc):
    """acc: [NRW, NCL] f32 psum dump -> dict of per-core partial sums."""
    a = acc.astype(np.float64).reshape(PK, NST, NMV, PK)
    v = np.einsum("fsmf->sm", a)          # [NST, NMV], diag over packed f
    T1 = np.array([v[c, c] for c in range(C)])
    T2 = np.array([v[c, C + c] for c in range(C)])
    L1 = v[C, 0:C]
    L2 = v[C, C:2 * C]
    B1 = v[C + 1, 0:C]
    B2 = v[C + 1, C:2 * C]
    bce_sum = v[C + 1, 2 * C]
    t_sum = v[C + 1, 2 * C + 1]
    bce_t_sum = v[C + 1, 2 * C + 2]
    return {
        "S1": T1 - L1, "S2": T2 - L2, "B1": B1, "B2": B2,
        "bce": bce_sum, "t": t_sum, "bce_t": bce_t_sum,
    }


def combine(acc_list):
    """acc_list: per-core [NRW, NCL] arrays -> final f32 scalar loss."""
    parts = [extract_core(a) for a in acc_list]

    seg_loss = 0.0
    att_loss = 0.0
    for n in range(N):
        pa, pb = parts[2 * n], parts[2 * n + 1]
        S1 = pa["S1"] + pb["S1"]
        S2 = pa["S2"] + pb["S2"]
        bins = pa["B1"] + pb["B1"]
        bins2 = pa["B2"] + pb["B2"]

        w1 = (bins != 0) * (1.0 - bins / HW) + 1.0
        seg_loss += -(w1 * S1).sum() / (w1 * bins).sum()

        vsum = bins2.sum()
        w2 = (bins2 != 0) * (1.0 - bins2 / vsum) + 1.0
        att_loss += -(w2 * S2).sum() / (w2 * bins2).sum()

    pos_bce = sum(p["bce_t"] for p in parts)
    all_bce = sum(p["bce"] for p in parts)
    pos_num = sum(p["t"] for p in parts)
    cnt = float(N * HW)
    neg_num = cnt - pos_num
    neg_bce = all_bce - pos_bce
    ssum = pos_num + neg_num
    edge_loss = (neg_num / ssum * pos_bce + pos_num / ssum * neg_bce) / cnt

    return np.float32(seg_loss + 0.3 * edge_loss + 0.1 * att_loss)


def run_cores(in_maps, trace=False, **kw):
    nc = _get_program()
    res = bass_utils.run_bass_kernel_spmd(
        nc, in_maps, core_ids=list(range(NCORES)), trace=trace, **kw
    )
    return res


def kernel(segin, edgein, segmask, edgemask):
    in_maps = make_in_maps(
        np.asarray(segin), np.asarray(edgein),
        np.asarray(segmask), np.asarray(edgemask))
    res = run_cores(in_maps)
    acc_list = [out["acc"] for out in res.results]
    return combine(acc_list)
